# revision 1
# baseline (speedup 1.0000x reference)
"""Trainium2 Bass kernel for nn_KnowledgeBaseLookup.

Computation (see reference):
    lookup = knowledge_base[indexes]            # (B,T,K,D) gather
    y      = einsum('btk,btkd->btd', weights, lookup)
    out    = y @ w_out.T + b_out                # (B,T,E)

Sharding: data-parallel over the B*T token dim across 8 cores; the
knowledge_base table is replicated per core.

Per-core layout (core owns 1024 tokens = 16384 gathered rows):
  - tokens are processed in 8 groups of 128 tokens (2048 rows).
  - one indirect DMA per group gathers 2048 rows of the table into a
    [128, 16*256] SBUF tile: slab s, partition p holds row (s*128+p).
  - stage 1 (weighted sum, transposed output): for each slab s and each
    128-wide d-chunk c, matmul(lhsT=gathered[:, s,c-chunk] [128,128],
    rhs=weight-mask [128,8]) -> yT_psum[c][:, s*8:(s+1)*8].  The weight
    mask column j carries weights[row] iff row's token == s*8+j, so the
    PE does the "multiply by weight + reduce over K=16" in one pass and
    produces y TRANSPOSED (d on partitions) - which stage 2 needs.
  - stage 2 (out_proj): matmul(lhsT=yT[:, c] [128d,128tok],
    rhs=w_out.T chunk [128d, 512e]) accumulated over the 2 d-chunks
    -> out_psum [128 tok, 512 e]; add bias; DMA out.
"""

import numpy as np

B, T, K = 4, 2048, 16
C, D, E = 262144, 256, 512
NCORES = 8
NTOK = B * T                      # 8192 tokens
TPC = NTOK // NCORES              # 1024 tokens per core
P = 128
GROUPS = TPC // P                 # 8 groups of 128 tokens per core
SLABS = (P * K) // P              # 16 slabs of 128 rows per group

_CACHE = {}


def _build_bass(repeats=1):
    import concourse.bass as bass
    import concourse.mybir as mybir
    from concourse import bacc
    from concourse.tile import TileContext

    fp32 = mybir.dt.float32
    nc = bacc.Bacc(
        "TRN2", target_bir_lowering=False, debug=False, num_devices=NCORES
    )

    kb = nc.dram_tensor("kb", [C, D], fp32, kind="ExternalInput")
    idx = nc.dram_tensor("idx", [P, GROUPS * SLABS], mybir.dt.int32,
                         kind="ExternalInput")
    wmask = nc.dram_tensor("wmask", [P, GROUPS * P], fp32, kind="ExternalInput")
    wout = nc.dram_tensor("wout", [P, 2 * E], fp32, kind="ExternalInput")
    bias = nc.dram_tensor("bias", [P, E], fp32, kind="ExternalInput")
    out = nc.dram_tensor("out", [TPC, E], fp32, kind="ExternalOutput")

    with TileContext(nc) as tc:
        with (
            tc.tile_pool(name="const", bufs=1) as cpool,
            tc.tile_pool(name="gather", bufs=2) as gpool,
            tc.tile_pool(name="y", bufs=2) as ypool,
            tc.tile_pool(name="osb", bufs=2) as opool,
            tc.tile_pool(name="psy", bufs=2, space="PSUM") as psy,
            tc.tile_pool(name="pso", bufs=2, space="PSUM") as pso,
        ):
            idx_sb = cpool.tile([P, GROUPS * SLABS], mybir.dt.int32)
            nc.sync.dma_start(out=idx_sb[:], in_=idx[:, :])
            wm_sb = cpool.tile([P, GROUPS * P], fp32)
            nc.sync.dma_start(out=wm_sb[:], in_=wmask[:, :])
            wo_sb = cpool.tile([P, 2 * E], fp32)
            nc.sync.dma_start(out=wo_sb[:], in_=wout[:, :])
            b_sb = cpool.tile([P, E], fp32)
            nc.sync.dma_start(out=b_sb[:], in_=bias[:, :])

            for g in range(GROUPS * repeats):
                g = g % GROUPS
                gath = gpool.tile([P, SLABS * D], fp32, tag="gath")
                # HW indirect DMA consumes ONE index per partition: gather
                # each 128-row slab with its own call (offset AP [128, 1],
                # dest [128, 256]).
                for s in range(SLABS):
                    col = g * SLABS + s
                    nc.gpsimd.indirect_dma_start(
                        out=gath[:, s * D:(s + 1) * D],
                        out_offset=None,
                        in_=kb[:, :],
                        in_offset=bass.IndirectOffsetOnAxis(
                            ap=idx_sb[:, col:col + 1], axis=0
                        ),
                    )

                yt0 = psy.tile([P, P], fp32, tag="yt0")
                yt1 = psy.tile([P, P], fp32, tag="yt1")
                for s in range(SLABS):
                    for c2, yt in enumerate((yt0, yt1)):
                        nc.tensor.matmul(
                            out=yt[:, s * 8:(s + 1) * 8],
                            lhsT=gath[:, s * D + c2 * P: s * D + (c2 + 1) * P],
                            rhs=wm_sb[:, g * P + s * 8: g * P + (s + 1) * 8],
                            start=True,
                            stop=True,
                        )

                y_sb = ypool.tile([P, D], fp32, tag="y")
                nc.vector.tensor_copy(out=y_sb[:, 0:P], in_=yt0[:])
                nc.vector.tensor_copy(out=y_sb[:, P:D], in_=yt1[:])

                o_ps = pso.tile([P, E], fp32, tag="ops")
                for c2 in range(2):
                    nc.tensor.matmul(
                        out=o_ps[:],
                        lhsT=y_sb[:, c2 * P:(c2 + 1) * P],
                        rhs=wo_sb[:, c2 * E:(c2 + 1) * E],
                        start=(c2 == 0),
                        stop=(c2 == 1),
                    )

                o_sb = opool.tile([P, E], fp32, tag="o")
                nc.vector.tensor_add(out=o_sb[:], in0=o_ps[:], in1=b_sb[:])
                nc.sync.dma_start(out=out[g * P:(g + 1) * P, :], in_=o_sb[:])

    nc.compile()
    return nc


def _host_prep(weights, indexes, w_out, b_out):
    """Build per-core input maps (everything except the replicated table)."""
    wflat = np.ascontiguousarray(weights, dtype=np.float32).reshape(NTOK, K)
    iflat = np.ascontiguousarray(indexes).reshape(NTOK, K).astype(np.int32)

    # w_out.T split into two 128-row d-chunks, chunk-major along free dim
    woutT = np.ascontiguousarray(w_out, dtype=np.float32).T  # [D, E]
    wout_host = np.ascontiguousarray(
        woutT.reshape(2, P, E).transpose(1, 0, 2).reshape(P, 2 * E)
    )
    bias_host = np.ascontiguousarray(
        np.broadcast_to(np.asarray(b_out, dtype=np.float32), (P, E))
    )
    # mask[p, j] = 1 iff partition p belongs to slab-local token j
    mask = (np.arange(P)[:, None] // K == np.arange(8)[None, :]).astype(np.float32)

    in_maps = []
    for c in range(NCORES):
        ic = iflat[c * TPC:(c + 1) * TPC].reshape(GROUPS, SLABS, P)
        wc = wflat[c * TPC:(c + 1) * TPC].reshape(GROUPS, SLABS, P)
        idx_host = np.ascontiguousarray(
            ic.transpose(2, 0, 1).reshape(P, GROUPS * SLABS)
        )
        w4 = wc[:, :, :, None] * mask[None, None, :, :]      # [G, S, P, 8]
        wmask_host = np.ascontiguousarray(
            w4.transpose(2, 0, 1, 3).reshape(P, GROUPS * P)
        )
        in_maps.append({
            "idx": idx_host,
            "wmask": wmask_host,
            "wout": wout_host,
            "bias": bias_host,
        })
    return in_maps


def kernel(weights, indexes, knowledge_base, w_out, b_out):
    from concourse.bass_utils import run_bass_kernel_spmd

    if "nc" not in _CACHE:
        _CACHE["nc"] = _build_bass()
    nc = _CACHE["nc"]

    kb_host = np.ascontiguousarray(knowledge_base, dtype=np.float32)
    in_maps = _host_prep(weights, indexes, w_out, b_out)
    for m in in_maps:
        m["kb"] = kb_host

    res = run_bass_kernel_spmd(nc, in_maps, list(range(NCORES)))
    out = np.concatenate([res.results[c]["out"] for c in range(NCORES)], axis=0)
    return out.reshape(B, T, E)



# revision 8
# speedup vs baseline: 2.0634x; 2.0634x over previous
"""Trainium2 Bass kernel for nn_KnowledgeBaseLookup.

Computation (see reference):
    lookup = knowledge_base[indexes]            # (B,T,K,D) gather
    y      = einsum('btk,btkd->btd', weights, lookup)
    out    = y @ w_out.T + b_out                # (B,T,E)

Sharding: data-parallel over the B*T token dim across 8 cores; the
knowledge_base table is replicated per core.

Per-core plan (1024 tokens = 16384 gathered rows):
  - The batched SWDGE gather (dma_gather / InstDMAGatherAnt) takes int16
    indexes, so the 262144-row table is addressed as 8 segments of 32768
    rows.  Host-side, each core's 16384 rows are bucketed by
    (token-half, segment) and token-sorted inside each bucket; one
    dma_gather per bucket (16 calls) moves exactly the needed rows.
  - Gathered slot i of a call lands at SBUF partition i%128, slab i//128.
    A slab's 128 token-sorted rows span a narrow token window; stage 1
    multiplies each slab (bitwise fp32) against a host-built [128, W]
    weight mask (W = token span) on the PE, accumulating yT[d, tok] into
    a per-half [128, 512] PSUM tile per d-chunk (PSUM pre-zeroed by DVE,
    all matmuls accumulate with start=False).
  - Masks ship as bf16 and are widened to fp32 on the DVE (matmul
    requires fp32 x fp32).
  - Drain per 128-token group: DVE copies yT PSUM -> SBUF as bf16,
    stage 2 (out_proj) runs as bf16 x bf16 matmuls over w_out.T chunks,
    bias is added on the DVE with bf16 output, and results DMA out as
    bf16 (host widens to fp32).

The program structure depends on the index data (bucket sizes, slab
windows), so kernel() compiles one program per core and caches them by
layout signature; core 0's program is kept in _CACHE["nc"] for timing.
"""

import numpy as np

B, T, K = 4, 2048, 16
C, D, E = 262144, 256, 512
NCORES = 8
NTOK = B * T                      # 8192 tokens
TPC = NTOK // NCORES              # 1024 tokens per core
P = 128
SEG = 32768                       # rows per int16-addressable table segment
NSEG = C // SEG                   # 8 segments
NHALF = 2                         # token halves per core (512 tokens each)
HTOK = TPC // NHALF
NGRP = TPC // P                   # 8 groups of 128 tokens per core

_CACHE = {}


def _ceil(a, b):
    return -(-a // b)


def _plan_core(idx_flat, w_flat):
    """Bucket one core's rows by (token-half, segment); build the gather
    index arrays and per-slab mask windows.  Returns a dict consumed by
    _build_bass (structure) and carrying the host tensors (data)."""
    t = np.arange(TPC * K, dtype=np.int64) // K
    half = t // HTOK
    seg = (idx_flat >> 15).astype(np.int64)
    order = np.lexsort((np.arange(TPC * K), seg, half))

    calls = []
    xoff = woff = 0
    for h in range(NHALF):
        for s in range(NSEG):
            sel = order[(half[order] == h) & (seg[order] == s)]
            n = len(sel)
            assert n > 0, (h, s)
            X, S = _ceil(n, 16), _ceil(n, 128)
            local = (idx_flat[sel] - s * SEG).astype(np.int16)
            idx16 = np.zeros((P, X), dtype=np.int16)
            pos = np.arange(n)
            for grp in range(8):
                idx16[pos % 16 + 16 * grp, pos // 16] = local
            tloc = (t[sel] - h * HTOK).astype(np.int64)
            slabs = []
            for j in range(S):
                rows = slice(128 * j, min(128 * j + 128, n))
                tj = tloc[rows]
                w0 = int(tj.min())
                W = int(tj.max()) - w0 + 1
                mask = np.zeros((P, W), dtype=np.float32)
                mask[np.arange(rows.stop - rows.start), tj - w0] = w_flat[sel[rows]]
                slabs.append({"w0": w0, "W": W, "woff": woff, "mask": mask})
                woff += W
            calls.append({
                "h": h, "seg": s, "n": n, "X": X, "S": S,
                "xoff": xoff, "idx16": idx16, "slabs": slabs,
            })
            xoff += X
    return {"calls": calls, "XTOT": xoff, "TOTW": woff,
            "SMAX": max(c["S"] for c in calls)}


def _plan_signature(plan):
    sig = []
    for c in plan["calls"]:
        sig.append((c["n"], tuple((s["w0"], s["W"]) for s in c["slabs"])))
    return tuple(sig)


def _build_bass(plan):
    import os

    import concourse.mybir as mybir
    from concourse import bacc
    from concourse.tile import TileContext

    gbufs = int(os.environ.get("K_GBUFS", "4"))
    do_widen = int(os.environ.get("K_WIDEN", "1"))
    do_stage1 = int(os.environ.get("K_STAGE1", "1"))
    do_drain = int(os.environ.get("K_DRAIN", "1"))
    ngath = int(os.environ.get("K_NGATH", "16"))

    fp32 = mybir.dt.float32
    bf16 = mybir.dt.bfloat16
    nc = bacc.Bacc(
        "TRN2", target_bir_lowering=False, debug=False,
        num_devices=NCORES, dynamic_dma_scratch_size=32768,
    )

    XTOT, TOTW, SMAX = plan["XTOT"], plan["TOTW"], plan["SMAX"]
    kb = nc.dram_tensor("kb", [C, D], fp32, kind="ExternalInput")
    idx = nc.dram_tensor("idx", [P, XTOT], mybir.dt.int16, kind="ExternalInput")
    wmb = nc.dram_tensor("wmb", [P, TOTW], bf16, kind="ExternalInput")
    wout = nc.dram_tensor("wout", [P, 2 * E], bf16, kind="ExternalInput")
    bias = nc.dram_tensor("bias", [P, E], fp32, kind="ExternalInput")
    out = nc.dram_tensor("out", [TPC, E], bf16, kind="ExternalOutput")

    with TileContext(nc) as tc:
        with (
            tc.tile_pool(name="const", bufs=1) as cpool,
            tc.tile_pool(name="gather", bufs=gbufs) as gpool,
            tc.tile_pool(name="y", bufs=2) as ypool,
            tc.tile_pool(name="osb", bufs=2) as opool,
            tc.tile_pool(name="psy", bufs=1, space="PSUM") as psy,
            tc.tile_pool(name="pso", bufs=2, space="PSUM") as pso,
        ):
            idx_sb = cpool.tile([P, XTOT], mybir.dt.int16)
            nc.sync.dma_start(out=idx_sb[:], in_=idx[:, :])
            wmb_sb = cpool.tile([P, TOTW], bf16)
            nc.sync.dma_start(out=wmb_sb[:], in_=wmb[:, :])
            wo_sb = cpool.tile([P, 2 * E], bf16)
            nc.sync.dma_start(out=wo_sb[:], in_=wout[:, :])
            b_sb = cpool.tile([P, E], fp32)
            nc.sync.dma_start(out=b_sb[:], in_=bias[:, :])

            # widen masks to fp32 per call-range on the DVE
            wm_sb = cpool.tile([P, TOTW], fp32)
            if do_widen:
                for c in plan["calls"]:
                    lo = c["slabs"][0]["woff"]
                    hi = c["slabs"][-1]["woff"] + c["slabs"][-1]["W"]
                    nc.vector.tensor_copy(out=wm_sb[:, lo:hi], in_=wmb_sb[:, lo:hi])

            # per-(half, d-chunk) yT accumulators, pre-zeroed
            yt = [[psy.tile([P, HTOK], fp32, name=f"yt{h}{c2}")
                   for c2 in range(2)] for h in range(NHALF)]
            for h in range(NHALF):
                for c2 in range(2):
                    nc.vector.memset(yt[h][c2][:], 0)

            def drain(h):
                for g4 in range(NGRP // NHALF):
                    g = h * (NGRP // NHALF) + g4
                    y_sb = ypool.tile([P, D], bf16, tag="y")
                    for c2 in range(2):
                        nc.vector.tensor_copy(
                            out=y_sb[:, c2 * P:(c2 + 1) * P],
                            in_=yt[h][c2][:, g4 * P:(g4 + 1) * P],
                        )
                    o_ps = pso.tile([P, E], fp32, tag="ops")
                    for c2 in range(2):
                        nc.tensor.matmul(
                            out=o_ps[:],
                            lhsT=y_sb[:, c2 * P:(c2 + 1) * P],
                            rhs=wo_sb[:, c2 * E:(c2 + 1) * E],
                            start=(c2 == 0), stop=(c2 == 1),
                        )
                    o_sb = opool.tile([P, E], bf16, tag="o")
                    nc.vector.tensor_add(out=o_sb[:], in0=o_ps[:], in1=b_sb[:])
                    nc.sync.dma_start(out=out[g * P:(g + 1) * P, :], in_=o_sb[:])

            for ci, c in enumerate(plan["calls"]):
                if ci >= ngath:
                    break
                h, s, n, S = c["h"], c["seg"], c["n"], c["S"]
                gath = gpool.tile([P, SMAX * D], fp32, tag="gath")
                nc.gpsimd.dma_gather(
                    gath[:, 0:S * D].rearrange("p (s d) -> p s d", d=D),
                    kb[s * SEG:(s + 1) * SEG, :],
                    idx_sb[:, c["xoff"]:c["xoff"] + c["X"]],
                    n, n, D,
                    single_packet=False,
                )
                last_call_of_half = (s == NSEG - 1)
                if not do_stage1:
                    continue
                for j, sl in enumerate(c["slabs"]):
                    last = last_call_of_half and (j == S - 1)
                    for c2 in range(2):
                        nc.tensor.matmul(
                            out=yt[h][c2][:, sl["w0"]:sl["w0"] + sl["W"]],
                            lhsT=gath[:, j * D + c2 * P: j * D + c2 * P + P],
                            rhs=wm_sb[:, sl["woff"]:sl["woff"] + sl["W"]],
                            start=False, stop=last, skip_group_check=True,
                        )
                if last_call_of_half and do_drain:
                    drain(h)

    nc.compile()
    return nc


def _host_prep(weights, indexes, w_out, b_out):
    """Per-core plans + host input maps (everything except the table)."""
    import ml_dtypes

    wflat = np.ascontiguousarray(weights, dtype=np.float32).reshape(NTOK * K)
    iflat = np.ascontiguousarray(indexes).reshape(NTOK * K).astype(np.int64)

    woutT = np.ascontiguousarray(w_out, dtype=np.float32).T       # [D, E]
    wout_host = np.ascontiguousarray(
        woutT.reshape(2, P, E).transpose(1, 0, 2).reshape(P, 2 * E)
    ).astype(ml_dtypes.bfloat16)
    bias_host = np.ascontiguousarray(
        np.broadcast_to(np.asarray(b_out, dtype=np.float32), (P, E))
    )

    plans, in_maps = [], []
    for c in range(NCORES):
        lo, hi = c * TPC * K, (c + 1) * TPC * K
        plan = _plan_core(iflat[lo:hi], wflat[lo:hi])
        wmb_host = np.zeros((P, plan["TOTW"]), dtype=ml_dtypes.bfloat16)
        idx_host = np.zeros((P, plan["XTOT"]), dtype=np.int16)
        for call in plan["calls"]:
            idx_host[:, call["xoff"]:call["xoff"] + call["X"]] = call["idx16"]
            for sl in call["slabs"]:
                wmb_host[:, sl["woff"]:sl["woff"] + sl["W"]] = (
                    sl["mask"].astype(ml_dtypes.bfloat16))
        plans.append(plan)
        in_maps.append({
            "idx": idx_host,
            "wmb": wmb_host,
            "wout": wout_host,
            "bias": bias_host,
        })
    return plans, in_maps


def kernel(weights, indexes, knowledge_base, w_out, b_out):
    from concourse.bass_utils import run_bass_kernel_spmd

    kb_host = np.ascontiguousarray(knowledge_base, dtype=np.float32)
    plans, in_maps = _host_prep(weights, indexes, w_out, b_out)

    outs = []
    for c in range(NCORES):
        sig = ("v1", _plan_signature(plans[c]))
        if sig not in _CACHE:
            _CACHE[sig] = _build_bass(plans[c])
        nc = _CACHE[sig]
        if c == 0:
            _CACHE["nc"] = nc
        in_maps[c]["kb"] = kb_host
        res = run_bass_kernel_spmd(nc, [in_maps[c]], [0])
        outs.append(res.results[0]["out"].astype(np.float32))

    return np.concatenate(outs, axis=0).reshape(B, T, E)


# revision 16
# speedup vs baseline: 2.2723x; 1.1013x over previous
"""Trainium2 Bass kernel for nn_KnowledgeBaseLookup.

Computation (see reference):
    lookup = knowledge_base[indexes]            # (B,T,K,D) gather
    y      = einsum('btk,btkd->btd', weights, lookup)
    out    = y @ w_out.T + b_out                # (B,T,E)

Sharding: data-parallel over the B*T token dim across 8 cores; the
knowledge_base table is replicated per core.

Per-core plan (1024 tokens = 16384 gathered rows):
  - The batched SWDGE gather (dma_gather / InstDMAGatherAnt) takes int16
    indexes, so the 262144-row table is addressed as 8 segments of 32768
    rows.  Host-side, each core's 16384 rows are bucketed by
    (token-half, segment) and token-sorted inside each bucket; one
    dma_gather per bucket (16 calls) moves exactly the needed rows.
  - Gathered slot i of a call lands at SBUF partition i%128, slab i//128.
    A slab's 128 token-sorted rows span a narrow token window; stage 1
    multiplies each slab (bitwise fp32) against a host-built [128, W]
    weight mask (W = token span) on the PE, accumulating yT[d, tok] into
    a per-half [128, 512] PSUM tile per d-chunk (PSUM pre-zeroed by DVE,
    all matmuls accumulate with start=False).
  - Masks ship as bf16 and are widened to fp32 on the DVE (matmul
    requires fp32 x fp32).
  - Drain per 128-token group: DVE copies yT PSUM -> SBUF as bf16,
    stage 2 (out_proj) runs as bf16 x bf16 matmuls over w_out.T chunks,
    bias is added on the DVE with bf16 output, and results DMA out as
    bf16 (host widens to fp32).

The program structure depends on the index data (bucket sizes, slab
windows), so kernel() compiles one program per core and caches them by
layout signature; core 0's program is kept in _CACHE["nc"] for timing.
"""

import numpy as np

B, T, K = 4, 2048, 16
C, D, E = 262144, 256, 512
NCORES = 8
NTOK = B * T                      # 8192 tokens
TPC = NTOK // NCORES              # 1024 tokens per core
P = 128
SEG = 32768                       # rows per int16-addressable table segment
NSEG = C // SEG                   # 8 segments
NSECT = 4                         # token sections per core (256 tokens each)
STOK = TPC // NSECT
NGRP = TPC // P                   # 8 groups of 128 tokens per core

_CACHE = {}


def _ceil(a, b):
    return -(-a // b)


def _plan_core(idx_flat, w_flat):
    """Bucket one core's rows by (token-half, segment); build the gather
    index arrays and per-slab mask windows.  Returns a dict consumed by
    _build_bass (structure) and carrying the host tensors (data)."""
    t = np.arange(TPC * K, dtype=np.int64) // K
    sect = t // STOK
    seg = (idx_flat >> 15).astype(np.int64)
    order = np.lexsort((np.arange(TPC * K), seg, sect))

    calls = []
    xoff = woff = 0
    for h in range(NSECT):
        for s in range(NSEG):
            sel = order[(sect[order] == h) & (seg[order] == s)]
            n = len(sel)
            assert n > 0, (h, s)
            X, S = _ceil(n, 16), _ceil(n, 128)
            local = (idx_flat[sel] - s * SEG).astype(np.int16)
            idx16 = np.zeros((P, X), dtype=np.int16)
            pos = np.arange(n)
            for grp in range(8):
                idx16[pos % 16 + 16 * grp, pos // 16] = local
            tloc = (t[sel] - h * STOK).astype(np.int64)
            slabs = []
            for j in range(S):
                rows = slice(128 * j, min(128 * j + 128, n))
                tj = tloc[rows]
                w0 = int(tj.min())
                W = int(tj.max()) - w0 + 1
                nr = rows.stop - rows.start
                tcol = np.full((P,), -1.0, dtype=np.float32)
                tcol[:nr] = tj.astype(np.float32)
                wcol = np.zeros((P,), dtype=np.float32)
                wcol[:nr] = w_flat[sel[rows]]
                slabs.append({"w0": w0, "W": W, "woff": woff,
                              "tcol": tcol, "wcol": wcol})
                woff += W
            calls.append({
                "h": h, "seg": s, "n": n, "X": X, "S": S,
                "xoff": xoff, "idx16": idx16, "slabs": slabs,
            })
            xoff += X
    return {"calls": calls, "XTOT": xoff, "TOTW": woff,
            "SMAX": max(c["S"] for c in calls)}


def _plan_signature(plan):
    sig = []
    for c in plan["calls"]:
        sig.append((c["n"], tuple((s["w0"], s["W"]) for s in c["slabs"])))
    return tuple(sig)


def _build_bass(plan):
    import os

    import concourse.mybir as mybir
    from concourse import bacc
    from concourse.tile import TileContext

    gbufs = int(os.environ.get("K_GBUFS", "4"))

    fp32 = mybir.dt.float32
    bf16 = mybir.dt.bfloat16
    fp8 = mybir.dt.float8e4
    nc = bacc.Bacc(
        "TRN2", target_bir_lowering=False, debug=False,
        num_devices=NCORES, dynamic_dma_scratch_size=32768,
    )

    XTOT, TOTW, SMAX = plan["XTOT"], plan["TOTW"], plan["SMAX"]
    NSLAB = plan["NSLAB"]
    kb = nc.dram_tensor("kb", [C, D], fp32, kind="ExternalInput")
    idx = nc.dram_tensor("idx", [P, XTOT], mybir.dt.int16, kind="ExternalInput")
    tw = nc.dram_tensor("tw", [P, 2 * NSLAB], fp32, kind="ExternalInput")
    iota = nc.dram_tensor("iota", [P, STOK], fp32, kind="ExternalInput")
    wout = nc.dram_tensor("wout", [P, 2 * E], bf16, kind="ExternalInput")
    bias = nc.dram_tensor("bias", [P, E], fp32, kind="ExternalInput")
    out = nc.dram_tensor("out", [TPC, E], bf16, kind="ExternalOutput")
    alu = mybir.AluOpType

    with TileContext(nc) as tc:
        with (
            tc.tile_pool(name="const", bufs=1) as cpool,
            tc.tile_pool(name="gather", bufs=gbufs) as gpool,
            tc.tile_pool(name="y", bufs=4) as ypool,
            tc.tile_pool(name="osb", bufs=NGRP) as opool,
            tc.tile_pool(name="psy", bufs=1, space="PSUM") as psy,
            tc.tile_pool(name="pso", bufs=2, space="PSUM") as pso,
        ):
            idx_sb = cpool.tile([P, XTOT], mybir.dt.int16)
            nc.sync.dma_start(out=idx_sb[:], in_=idx[:, :])
            tw_sb = cpool.tile([P, 2 * NSLAB], fp32)
            nc.sync.dma_start(out=tw_sb[:], in_=tw[:, :])
            iota_sb = cpool.tile([P, STOK], fp32)
            nc.sync.dma_start(out=iota_sb[:], in_=iota[:, :])
            wo_sb = cpool.tile([P, 2 * E], bf16)
            nc.sync.dma_start(out=wo_sb[:], in_=wout[:, :])
            b_sb = cpool.tile([P, E], fp32)
            nc.sync.dma_start(out=b_sb[:], in_=bias[:, :])

            # generate fp32 masks on the DVE: mask[p, c] = w[p] * (iota == tloc[p])
            wm_sb = cpool.tile([P, TOTW], fp32)
            for c in plan["calls"]:
                for sl in c["slabs"]:
                    si, w0, W = sl["si"], sl["w0"], sl["W"]
                    nc.vector.tensor_scalar(
                        out=wm_sb[:, sl["woff"]:sl["woff"] + W],
                        in0=iota_sb[:, w0:w0 + W],
                        scalar1=tw_sb[:, 2 * si:2 * si + 1],
                        scalar2=tw_sb[:, 2 * si + 1:2 * si + 2],
                        op0=alu.is_equal, op1=alu.mult,
                    )

            # two parity-rotated yT accumulators per d-chunk
            yt = [[psy.tile([P, STOK], fp32, name=f"yt{par}{c2}")
                   for c2 in range(2)] for par in range(2)]

            pending_outs = []

            def drain(sect):
                par = sect % 2
                gpsect = NGRP // NSECT
                y_sbs = []
                for g4 in range(gpsect):
                    y_sb = ypool.tile([P, D], bf16, tag="y")
                    y_sbs.append(y_sb)
                    for c2 in range(2):
                        nc.scalar.copy(
                            out=y_sb[:, c2 * P:(c2 + 1) * P],
                            in_=yt[par][c2][:, g4 * P:(g4 + 1) * P],
                        )
                o_pss = []
                for g4 in range(gpsect):
                    o_ps = pso.tile([P, E], fp32, tag="ops")
                    o_pss.append(o_ps)
                    for c2 in range(2):
                        nc.tensor.matmul(
                            out=o_ps[:],
                            lhsT=y_sbs[g4][:, c2 * P:(c2 + 1) * P],
                            rhs=wo_sb[:, c2 * E:(c2 + 1) * E],
                            start=(c2 == 0), stop=(c2 == 1),
                        )
                for g4 in range(gpsect):
                    g = sect * gpsect + g4
                    o_sb = opool.tile([P, E], bf16, tag="o")
                    nc.vector.tensor_add(out=o_sb[:], in0=o_pss[g4][:], in1=b_sb[:])
                    pending_outs.append((g, o_sb))

            for ci, c in enumerate(plan["calls"]):
                h, s, n, S = c["h"], c["seg"], c["n"], c["S"]
                par = h % 2
                if s == 0:
                    for c2 in range(2):
                        nc.vector.memset(yt[par][c2][:], 0)
                gath = gpool.tile([P, SMAX * D], fp32, tag="gath")
                nc.gpsimd.dma_gather(
                    gath[:, 0:S * D].rearrange("p (s d) -> p s d", d=D),
                    kb[s * SEG:(s + 1) * SEG, :],
                    idx_sb[:, c["xoff"]:c["xoff"] + c["X"]],
                    n, n, D,
                    single_packet=False,
                )
                last_call_of_sect = (s == NSEG - 1)
                for j, sl in enumerate(c["slabs"]):
                    last = last_call_of_sect and (j == S - 1)
                    for c2 in range(2):
                        nc.tensor.matmul(
                            out=yt[par][c2][:, sl["w0"]:sl["w0"] + sl["W"]],
                            lhsT=gath[:, j * D + c2 * P: j * D + c2 * P + P],
                            rhs=wm_sb[:, sl["woff"]:sl["woff"] + sl["W"]],
                            start=False, stop=last, skip_group_check=True,
                        )
                if last_call_of_sect:
                    drain(h)

            # all output DMAs deferred behind the gather stream
            for g, o_sb in pending_outs:
                nc.sync.dma_start(out=out[g * P:(g + 1) * P, :], in_=o_sb[:])

    nc.compile()
    return nc


def _host_prep(weights, indexes, w_out, b_out):
    """Per-core plans + host input maps (everything except the table)."""
    import ml_dtypes

    wflat = np.ascontiguousarray(weights, dtype=np.float32).reshape(NTOK * K)
    iflat = np.ascontiguousarray(indexes).reshape(NTOK * K).astype(np.int64)

    woutT = np.ascontiguousarray(w_out, dtype=np.float32).T       # [D, E]
    wout_host = np.ascontiguousarray(
        woutT.reshape(2, P, E).transpose(1, 0, 2).reshape(P, 2 * E)
    ).astype(ml_dtypes.bfloat16)
    bias_host = np.ascontiguousarray(
        np.broadcast_to(np.asarray(b_out, dtype=np.float32), (P, E))
    )

    plans, in_maps = [], []
    for c in range(NCORES):
        lo, hi = c * TPC * K, (c + 1) * TPC * K
        plan = _plan_core(iflat[lo:hi], wflat[lo:hi])
        nslab = sum(c["S"] for c in plan["calls"])
        plan["NSLAB"] = nslab
        tw_host = np.zeros((P, 2 * nslab), dtype=np.float32)
        idx_host = np.zeros((P, plan["XTOT"]), dtype=np.int16)
        si = 0
        for call in plan["calls"]:
            idx_host[:, call["xoff"]:call["xoff"] + call["X"]] = call["idx16"]
            for sl in call["slabs"]:
                sl["si"] = si
                tw_host[:, 2 * si] = sl["tcol"]
                tw_host[:, 2 * si + 1] = sl["wcol"]
                si += 1
        iota_host = np.broadcast_to(
            np.arange(STOK, dtype=np.float32), (P, STOK)).copy()
        plans.append(plan)
        in_maps.append({
            "idx": idx_host,
            "tw": tw_host,
            "iota": iota_host,
            "wout": wout_host,
            "bias": bias_host,
        })
    return plans, in_maps


def kernel(weights, indexes, knowledge_base, w_out, b_out):
    from concourse.bass_utils import run_bass_kernel_spmd

    kb_host = np.ascontiguousarray(knowledge_base, dtype=np.float32)
    plans, in_maps = _host_prep(weights, indexes, w_out, b_out)

    outs = []
    for c in range(NCORES):
        sig = ("v4", _plan_signature(plans[c]))
        if sig not in _CACHE:
            _CACHE[sig] = _build_bass(plans[c])
        nc = _CACHE[sig]
        if c == 0:
            _CACHE["nc"] = nc
        in_maps[c]["kb"] = kb_host
        res = run_bass_kernel_spmd(nc, [in_maps[c]], [0])
        outs.append(res.results[0]["out"].astype(np.float32))

    return np.concatenate(outs, axis=0).reshape(B, T, E)


# revision 19
# speedup vs baseline: 2.3473x; 1.0330x over previous
"""Trainium2 Bass kernel for nn_KnowledgeBaseLookup.

Computation (see reference):
    lookup = knowledge_base[indexes]            # (B,T,K,D) gather
    y      = einsum('btk,btkd->btd', weights, lookup)
    out    = y @ w_out.T + b_out                # (B,T,E)

Sharding: data-parallel over the B*T token dim across 8 cores; the
knowledge_base table is replicated per core.

Per-core plan (1024 tokens = 16384 gathered rows):
  - The batched SWDGE gather (dma_gather / InstDMAGatherAnt) takes int16
    indexes, so the 262144-row table is addressed as 8 segments of 32768
    rows.  Host-side, each core's rows are bucketed by (token-section,
    segment) -- 4 sections x 8 segments = 32 dma_gather calls of ~512
    rows each, token-sorted inside each bucket, moving exactly the
    needed rows (no padding descriptors).
  - Gathered slot i of a call lands at SBUF partition i%128, slab
    i//128.  A slab's 128 token-sorted rows span a narrow token window;
    stage 1 multiplies each slab against a [128, W] fp32 weight mask on
    the PE, accumulating yT[d, tok] into parity-rotated [128, 256] PSUM
    tiles per d-chunk (DVE pre-zeroes them; matmuls use start=False).
  - Masks are GENERATED on the DVE (one fused is_equal+mult
    tensor_scalar per slab against an iota row and per-slab tloc/w
    columns), so only ~1.5 KB/partition of mask metadata is DMA'd.
  - Drain per 256-token section: ACT copies yT PSUM -> SBUF as bf16,
    stage 2 (out_proj) runs as bf16 x bf16 matmuls over w_out.T chunks,
    DVE adds bias with bf16 output, and results DMA out as bf16 (host
    widens to fp32).

The program structure depends on the index data (bucket sizes, slab
windows), so kernel() compiles one program per core and caches them by
layout signature; core 0's program is kept in _CACHE["nc"] for timing.
"""

import numpy as np

B, T, K = 4, 2048, 16
C, D, E = 262144, 256, 512
NCORES = 8
NTOK = B * T                      # 8192 tokens
TPC = NTOK // NCORES              # 1024 tokens per core
P = 128
SEG = 32768                       # rows per int16-addressable table segment
NSEG = C // SEG                   # 8 segments
NSECT = 4                         # token sections per core
# asymmetric: big sections early, tiny last section to shrink the drain tail
SECT_BOUNDS = (0, 384, 640, 896, 1024)
SMAXTOK = 384                     # widest section
NGRP = TPC // P                   # 8 groups of 128 tokens per core

_CACHE = {}


def _ceil(a, b):
    return -(-a // b)


def _plan_core(idx_flat, w_flat):
    """Bucket one core's rows by (token-half, segment); build the gather
    index arrays and per-slab mask windows.  Returns a dict consumed by
    _build_bass (structure) and carrying the host tensors (data)."""
    t = np.arange(TPC * K, dtype=np.int64) // K
    sect = np.searchsorted(np.asarray(SECT_BOUNDS), t, side="right") - 1
    seg = (idx_flat >> 15).astype(np.int64)
    order = np.lexsort((np.arange(TPC * K), seg, sect))

    calls = []
    xoff = woff = 0
    for h in range(NSECT):
        for s in range(NSEG):
            sel = order[(sect[order] == h) & (seg[order] == s)]
            n = len(sel)
            assert n > 0, (h, s)
            X, S = _ceil(n, 16), _ceil(n, 128)
            local = (idx_flat[sel] - s * SEG).astype(np.int16)
            idx16 = np.zeros((P, X), dtype=np.int16)
            pos = np.arange(n)
            for grp in range(8):
                idx16[pos % 16 + 16 * grp, pos // 16] = local
            tloc = (t[sel] - SECT_BOUNDS[h]).astype(np.int64)
            slabs = []
            for j in range(S):
                rows = slice(128 * j, min(128 * j + 128, n))
                tj = tloc[rows]
                w0 = int(tj.min())
                W = int(tj.max()) - w0 + 1
                nr = rows.stop - rows.start
                tcol = np.full((P,), -1.0, dtype=np.float32)
                tcol[:nr] = tj.astype(np.float32)
                wcol = np.zeros((P,), dtype=np.float32)
                wcol[:nr] = w_flat[sel[rows]]
                slabs.append({"w0": w0, "W": W, "woff": woff,
                              "tcol": tcol, "wcol": wcol})
                woff += W
            calls.append({
                "h": h, "seg": s, "n": n, "X": X, "S": S,
                "xoff": xoff, "idx16": idx16, "slabs": slabs,
            })
            xoff += X
    return {"calls": calls, "XTOT": xoff, "TOTW": woff,
            "SMAX": max(c["S"] for c in calls)}


def _plan_signature(plan):
    sig = []
    for c in plan["calls"]:
        sig.append((c["n"], tuple((s["w0"], s["W"]) for s in c["slabs"])))
    return tuple(sig)


def _build_bass(plan):
    import os

    import concourse.mybir as mybir
    from concourse import bacc
    from concourse.tile import TileContext

    gbufs = int(os.environ.get("K_GBUFS", "6"))

    fp32 = mybir.dt.float32
    bf16 = mybir.dt.bfloat16
    fp8 = mybir.dt.float8e4
    nc = bacc.Bacc(
        "TRN2", target_bir_lowering=False, debug=False,
        num_devices=NCORES, dynamic_dma_scratch_size=49152,
    )

    XTOT, TOTW, SMAX = plan["XTOT"], plan["TOTW"], plan["SMAX"]
    NSLAB = plan["NSLAB"]
    kb = nc.dram_tensor("kb", [C, D], fp32, kind="ExternalInput")
    idx = nc.dram_tensor("idx", [P, XTOT], mybir.dt.int16, kind="ExternalInput")
    tw = nc.dram_tensor("tw", [P, 2 * NSLAB], fp32, kind="ExternalInput")
    iota = nc.dram_tensor("iota", [P, SMAXTOK], fp32, kind="ExternalInput")
    wout = nc.dram_tensor("wout", [P, 2 * E], bf16, kind="ExternalInput")
    bias = nc.dram_tensor("bias", [P, E], fp32, kind="ExternalInput")
    out = nc.dram_tensor("out", [TPC, E], bf16, kind="ExternalOutput")
    alu = mybir.AluOpType

    with TileContext(nc) as tc:
        with (
            tc.tile_pool(name="const", bufs=1) as cpool,
            tc.tile_pool(name="gather", bufs=gbufs) as gpool,
            tc.tile_pool(name="y", bufs=4) as ypool,
            tc.tile_pool(name="osb", bufs=NGRP) as opool,
            tc.tile_pool(name="psy", bufs=1, space="PSUM") as psy,
            tc.tile_pool(name="pso", bufs=2, space="PSUM") as pso,
        ):
            idx_sb = cpool.tile([P, XTOT], mybir.dt.int16)
            nc.sync.dma_start(out=idx_sb[:], in_=idx[:, :])
            tw_sb = cpool.tile([P, 2 * NSLAB], fp32)
            nc.sync.dma_start(out=tw_sb[:], in_=tw[:, :])
            iota_sb = cpool.tile([P, SMAXTOK], fp32)
            nc.sync.dma_start(out=iota_sb[:], in_=iota[:, :])
            wo_sb = cpool.tile([P, 2 * E], bf16)
            nc.sync.dma_start(out=wo_sb[:], in_=wout[:, :])
            b_sb = cpool.tile([P, E], fp32)
            nc.sync.dma_start(out=b_sb[:], in_=bias[:, :])

            # generate fp32 masks on the DVE: mask[p, c] = w[p] * (iota == tloc[p])
            wm_sb = cpool.tile([P, TOTW], fp32)
            for c in plan["calls"]:
                for sl in c["slabs"]:
                    si, w0, W = sl["si"], sl["w0"], sl["W"]
                    nc.vector.tensor_scalar(
                        out=wm_sb[:, sl["woff"]:sl["woff"] + W],
                        in0=iota_sb[:, w0:w0 + W],
                        scalar1=tw_sb[:, 2 * si:2 * si + 1],
                        scalar2=tw_sb[:, 2 * si + 1:2 * si + 2],
                        op0=alu.is_equal, op1=alu.mult,
                    )

            # two parity-rotated yT accumulators per d-chunk
            yt = [[psy.tile([P, SMAXTOK], fp32, name=f"yt{par}{c2}")
                   for c2 in range(2)] for par in range(2)]

            pending_outs = []

            def drain(sect):
                par = sect % 2
                gbase = SECT_BOUNDS[sect] // P
                gpsect = (SECT_BOUNDS[sect + 1] - SECT_BOUNDS[sect]) // P
                y_sbs = []
                for g4 in range(gpsect):
                    y_sb = ypool.tile([P, D], bf16, tag="y")
                    y_sbs.append(y_sb)
                    for c2 in range(2):
                        nc.scalar.copy(
                            out=y_sb[:, c2 * P:(c2 + 1) * P],
                            in_=yt[par][c2][:, g4 * P:(g4 + 1) * P],
                        )
                o_pss = []
                for g4 in range(gpsect):
                    o_ps = pso.tile([P, E], fp32, tag="ops")
                    o_pss.append(o_ps)
                    for c2 in range(2):
                        nc.tensor.matmul(
                            out=o_ps[:],
                            lhsT=y_sbs[g4][:, c2 * P:(c2 + 1) * P],
                            rhs=wo_sb[:, c2 * E:(c2 + 1) * E],
                            start=(c2 == 0), stop=(c2 == 1),
                        )
                for g4 in range(gpsect):
                    g = gbase + g4
                    o_sb = opool.tile([P, E], bf16, tag="o")
                    nc.vector.tensor_add(out=o_sb[:], in0=o_pss[g4][:], in1=b_sb[:])
                    pending_outs.append((g, o_sb))

            for ci, c in enumerate(plan["calls"]):
                h, s, n, S = c["h"], c["seg"], c["n"], c["S"]
                par = h % 2
                if s == 0:
                    for c2 in range(2):
                        nc.vector.memset(yt[par][c2][:], 0)
                gath = gpool.tile([P, SMAX * D], fp32, tag="gath")
                nc.gpsimd.dma_gather(
                    gath[:, 0:S * D].rearrange("p (s d) -> p s d", d=D),
                    kb[s * SEG:(s + 1) * SEG, :],
                    idx_sb[:, c["xoff"]:c["xoff"] + c["X"]],
                    n, n, D,
                    single_packet=False,
                )
                last_call_of_sect = (s == NSEG - 1)
                for j, sl in enumerate(c["slabs"]):
                    last = last_call_of_sect and (j == S - 1)
                    for c2 in range(2):
                        nc.tensor.matmul(
                            out=yt[par][c2][:, sl["w0"]:sl["w0"] + sl["W"]],
                            lhsT=gath[:, j * D + c2 * P: j * D + c2 * P + P],
                            rhs=wm_sb[:, sl["woff"]:sl["woff"] + sl["W"]],
                            start=False, stop=last, skip_group_check=True,
                        )
                if last_call_of_sect:
                    drain(h)

            # all output DMAs deferred behind the gather stream
            for g, o_sb in pending_outs:
                nc.sync.dma_start(out=out[g * P:(g + 1) * P, :], in_=o_sb[:])

    nc.compile()
    return nc


def _host_prep(weights, indexes, w_out, b_out):
    """Per-core plans + host input maps (everything except the table)."""
    import ml_dtypes

    wflat = np.ascontiguousarray(weights, dtype=np.float32).reshape(NTOK * K)
    iflat = np.ascontiguousarray(indexes).reshape(NTOK * K).astype(np.int64)

    woutT = np.ascontiguousarray(w_out, dtype=np.float32).T       # [D, E]
    wout_host = np.ascontiguousarray(
        woutT.reshape(2, P, E).transpose(1, 0, 2).reshape(P, 2 * E)
    ).astype(ml_dtypes.bfloat16)
    bias_host = np.ascontiguousarray(
        np.broadcast_to(np.asarray(b_out, dtype=np.float32), (P, E))
    )

    plans, in_maps = [], []
    for c in range(NCORES):
        lo, hi = c * TPC * K, (c + 1) * TPC * K
        plan = _plan_core(iflat[lo:hi], wflat[lo:hi])
        nslab = sum(c["S"] for c in plan["calls"])
        plan["NSLAB"] = nslab
        tw_host = np.zeros((P, 2 * nslab), dtype=np.float32)
        idx_host = np.zeros((P, plan["XTOT"]), dtype=np.int16)
        si = 0
        for call in plan["calls"]:
            idx_host[:, call["xoff"]:call["xoff"] + call["X"]] = call["idx16"]
            for sl in call["slabs"]:
                sl["si"] = si
                tw_host[:, 2 * si] = sl["tcol"]
                tw_host[:, 2 * si + 1] = sl["wcol"]
                si += 1
        iota_host = np.broadcast_to(
            np.arange(SMAXTOK, dtype=np.float32), (P, SMAXTOK)).copy()
        plans.append(plan)
        in_maps.append({
            "idx": idx_host,
            "tw": tw_host,
            "iota": iota_host,
            "wout": wout_host,
            "bias": bias_host,
        })
    return plans, in_maps


def kernel(weights, indexes, knowledge_base, w_out, b_out):
    from concourse.bass_utils import run_bass_kernel_spmd

    kb_host = np.ascontiguousarray(knowledge_base, dtype=np.float32)
    plans, in_maps = _host_prep(weights, indexes, w_out, b_out)

    outs = []
    for c in range(NCORES):
        sig = ("v6", _plan_signature(plans[c]))
        if sig not in _CACHE:
            _CACHE[sig] = _build_bass(plans[c])
        nc = _CACHE[sig]
        if c == 0:
            _CACHE["nc"] = nc
        in_maps[c]["kb"] = kb_host
        res = run_bass_kernel_spmd(nc, [in_maps[c]], [0])
        outs.append(res.results[0]["out"].astype(np.float32))

    return np.concatenate(outs, axis=0).reshape(B, T, E)


# revision 24
# speedup vs baseline: 2.3948x; 1.0203x over previous
"""Trainium2 Bass kernel for nn_KnowledgeBaseLookup.

Computation (see reference):
    lookup = knowledge_base[indexes]            # (B,T,K,D) gather
    y      = einsum('btk,btkd->btd', weights, lookup)
    out    = y @ w_out.T + b_out                # (B,T,E)

Sharding: data-parallel over the B*T token dim across 8 cores; the
knowledge_base table is replicated per core.

Per-core plan (1024 tokens = 16384 gathered rows):
  - The batched SWDGE gather (dma_gather / InstDMAGatherAnt) takes int16
    indexes, so the 262144-row table is addressed as 8 segments of 32768
    rows.  Host-side, each core's rows are bucketed by (token-section,
    segment) -- 4 sections x 8 segments = 32 dma_gather calls of ~512
    rows each, token-sorted inside each bucket, moving exactly the
    needed rows (no padding descriptors).
  - Gathered slot i of a call lands at SBUF partition i%128, slab
    i//128.  A slab's 128 token-sorted rows span a narrow token window;
    stage 1 multiplies each slab against a [128, W] fp32 weight mask on
    the PE, accumulating yT[d, tok] into parity-rotated [128, 256] PSUM
    tiles per d-chunk (DVE pre-zeroes them; matmuls use start=False).
  - Masks are GENERATED on the DVE (one fused is_equal+mult
    tensor_scalar per slab against an iota row and per-slab tloc/w
    columns), so only ~1.5 KB/partition of mask metadata is DMA'd.
  - Drain per 256-token section: ACT copies yT PSUM -> SBUF as bf16,
    stage 2 (out_proj) runs as bf16 x bf16 matmuls over w_out.T chunks,
    DVE adds bias with bf16 output, and results DMA out as bf16 (host
    widens to fp32).

The program structure depends on the index data (bucket sizes, slab
windows), so kernel() compiles one program per core and caches them by
layout signature; core 0's program is kept in _CACHE["nc"] for timing.
"""

import numpy as np

B, T, K = 4, 2048, 16
C, D, E = 262144, 256, 512
NCORES = 8
NTOK = B * T                      # 8192 tokens
TPC = NTOK // NCORES              # 1024 tokens per core
P = 128
SEG = 32768                       # rows per int16-addressable table segment
NSEG = C // SEG                   # 8 segments
NSECT = 4                         # token sections per core
# asymmetric: big sections early, tiny last section to shrink the drain tail
SECT_BOUNDS = (0, 384, 640, 896, 1024)
SMAXTOK = 384                     # widest section
NGRP = TPC // P                   # 8 groups of 128 tokens per core

_CACHE = {}


def _ceil(a, b):
    return -(-a // b)


def _plan_core(idx_flat, w_flat):
    """Bucket one core's rows by (token-half, segment); build the gather
    index arrays and per-slab mask windows.  Returns a dict consumed by
    _build_bass (structure) and carrying the host tensors (data)."""
    t = np.arange(TPC * K, dtype=np.int64) // K
    sect = np.searchsorted(np.asarray(SECT_BOUNDS), t, side="right") - 1
    seg = (idx_flat >> 15).astype(np.int64)
    order = np.lexsort((np.arange(TPC * K), seg, sect))

    calls = []
    xoff = woff = 0
    for h in range(NSECT):
        for s in range(NSEG):
            sel = order[(sect[order] == h) & (seg[order] == s)]
            n = len(sel)
            assert n > 0, (h, s)
            X, S = _ceil(n, 16), _ceil(n, 128)
            local = (idx_flat[sel] - s * SEG).astype(np.int16)
            idx16 = np.zeros((P, X), dtype=np.int16)
            pos = np.arange(n)
            for grp in range(8):
                idx16[pos % 16 + 16 * grp, pos // 16] = local
            tloc = (t[sel] - SECT_BOUNDS[h]).astype(np.int64)
            slabs = []
            for j in range(S):
                rows = slice(128 * j, min(128 * j + 128, n))
                tj = tloc[rows]
                w0 = int(tj.min())
                W = int(tj.max()) - w0 + 1
                nr = rows.stop - rows.start
                tcol = np.full((P,), -1.0, dtype=np.float32)
                tcol[:nr] = tj.astype(np.float32)
                wcol = np.zeros((P,), dtype=np.float32)
                wcol[:nr] = w_flat[sel[rows]]
                slabs.append({"w0": w0, "W": W, "woff": woff,
                              "tcol": tcol, "wcol": wcol})
                woff += W
            calls.append({
                "h": h, "seg": s, "n": n, "X": X, "S": S,
                "xoff": xoff, "idx16": idx16, "slabs": slabs,
            })
            xoff += X
    return {"calls": calls, "XTOT": xoff, "TOTW": woff,
            "SMAX": max(c["S"] for c in calls)}


def _plan_signature(plan):
    sig = []
    for c in plan["calls"]:
        sig.append((c["n"], tuple((s["w0"], s["W"]) for s in c["slabs"])))
    return tuple(sig)


def _build_bass(plan):
    import os

    import concourse.mybir as mybir
    from concourse import bacc
    from concourse.tile import TileContext

    gbufs = int(os.environ.get("K_GBUFS", "8"))

    fp32 = mybir.dt.float32
    bf16 = mybir.dt.bfloat16
    fp8 = mybir.dt.float8e4
    nc = bacc.Bacc(
        "TRN2", target_bir_lowering=False, debug=False,
        num_devices=NCORES, dynamic_dma_scratch_size=49152,
    )

    XTOT, TOTW, SMAX = plan["XTOT"], plan["TOTW"], plan["SMAX"]
    NSLAB = plan["NSLAB"]
    kb = nc.dram_tensor("kb", [C, D], fp32, kind="ExternalInput")
    idx = nc.dram_tensor("idx", [P, XTOT], mybir.dt.int16, kind="ExternalInput")
    tw = nc.dram_tensor("tw", [P, 2 * NSLAB], fp32, kind="ExternalInput")
    iota = nc.dram_tensor("iota", [P, SMAXTOK], fp32, kind="ExternalInput")
    wout = nc.dram_tensor("wout", [P, 2 * E], bf16, kind="ExternalInput")
    bias = nc.dram_tensor("bias", [P, E], fp32, kind="ExternalInput")
    out = nc.dram_tensor("out", [TPC, E], bf16, kind="ExternalOutput")
    alu = mybir.AluOpType

    with TileContext(nc) as tc:
        with (
            tc.tile_pool(name="const", bufs=1) as cpool,
            tc.tile_pool(name="gather", bufs=gbufs) as gpool,
            tc.tile_pool(name="y", bufs=4) as ypool,
            tc.tile_pool(name="osb", bufs=NGRP) as opool,
            tc.tile_pool(name="psy", bufs=1, space="PSUM") as psy,
            tc.tile_pool(name="pso", bufs=2, space="PSUM") as pso,
        ):
            idx_sb = cpool.tile([P, XTOT], mybir.dt.int16)
            nc.sync.dma_start(out=idx_sb[:], in_=idx[:, :])
            tw_sb = cpool.tile([P, 2 * NSLAB], fp32)
            nc.sync.dma_start(out=tw_sb[:], in_=tw[:, :])
            iota_sb = cpool.tile([P, SMAXTOK], fp32)
            nc.sync.dma_start(out=iota_sb[:], in_=iota[:, :])
            wo_sb = cpool.tile([P, 2 * E], bf16)
            nc.sync.dma_start(out=wo_sb[:], in_=wout[:, :])
            b_sb = cpool.tile([P, E], fp32)
            nc.sync.dma_start(out=b_sb[:], in_=bias[:, :])

            # generate fp32 masks on the DVE: mask[p, c] = w[p] * (iota == tloc[p])
            wm_sb = cpool.tile([P, TOTW], fp32)
            for c in plan["calls"]:
                for sl in c["slabs"]:
                    si, w0, W = sl["si"], sl["w0"], sl["W"]
                    nc.vector.tensor_scalar(
                        out=wm_sb[:, sl["woff"]:sl["woff"] + W],
                        in0=iota_sb[:, w0:w0 + W],
                        scalar1=tw_sb[:, 2 * si:2 * si + 1],
                        scalar2=tw_sb[:, 2 * si + 1:2 * si + 2],
                        op0=alu.is_equal, op1=alu.mult,
                    )

            # two parity-rotated yT accumulators per d-chunk
            yt = [[psy.tile([P, SMAXTOK], fp32, name=f"yt{par}{c2}")
                   for c2 in range(2)] for par in range(2)]

            pending_outs = []

            def drain(sect):
                par = sect % 2
                gbase = SECT_BOUNDS[sect] // P
                gpsect = (SECT_BOUNDS[sect + 1] - SECT_BOUNDS[sect]) // P
                y_sbs = []
                for g4 in range(gpsect):
                    y_sb = ypool.tile([P, D], bf16, tag="y")
                    y_sbs.append(y_sb)
                    for c2 in range(2):
                        nc.scalar.copy(
                            out=y_sb[:, c2 * P:(c2 + 1) * P],
                            in_=yt[par][c2][:, g4 * P:(g4 + 1) * P],
                        )
                o_pss = []
                for g4 in range(gpsect):
                    o_ps = pso.tile([P, E], fp32, tag="ops")
                    o_pss.append(o_ps)
                    for c2 in range(2):
                        nc.tensor.matmul(
                            out=o_ps[:],
                            lhsT=y_sbs[g4][:, c2 * P:(c2 + 1) * P],
                            rhs=wo_sb[:, c2 * E:(c2 + 1) * E],
                            start=(c2 == 0), stop=(c2 == 1),
                        )
                for g4 in range(gpsect):
                    g = gbase + g4
                    o_sb = opool.tile([P, E], bf16, tag="o")
                    nc.vector.tensor_add(out=o_sb[:], in0=o_pss[g4][:], in1=b_sb[:])
                    pending_outs.append((g, o_sb))

            for ci, c in enumerate(plan["calls"]):
                h, s, n, S = c["h"], c["seg"], c["n"], c["S"]
                par = h % 2
                if s == 0:
                    for c2 in range(2):
                        nc.vector.memset(yt[par][c2][:], 0)
                gath = gpool.tile([P, SMAX * D], fp32, tag="gath")
                nc.gpsimd.dma_gather(
                    gath[:, 0:S * D].rearrange("p (s d) -> p s d", d=D),
                    kb[s * SEG:(s + 1) * SEG, :],
                    idx_sb[:, c["xoff"]:c["xoff"] + c["X"]],
                    n, n, D,
                    single_packet=False,
                )
                last_call_of_sect = (s == NSEG - 1)
                for j, sl in enumerate(c["slabs"]):
                    last = last_call_of_sect and (j == S - 1)
                    for c2 in range(2):
                        nc.tensor.matmul(
                            out=yt[par][c2][:, sl["w0"]:sl["w0"] + sl["W"]],
                            lhsT=gath[:, j * D + c2 * P: j * D + c2 * P + P],
                            rhs=wm_sb[:, sl["woff"]:sl["woff"] + sl["W"]],
                            start=False, stop=last, skip_group_check=True,
                        )
                if last_call_of_sect:
                    drain(h)

            # all output DMAs deferred behind the gather stream
            for g, o_sb in pending_outs:
                nc.sync.dma_start(out=out[g * P:(g + 1) * P, :], in_=o_sb[:])

    nc.compile()
    return nc


def _host_prep(weights, indexes, w_out, b_out):
    """Per-core plans + host input maps (everything except the table)."""
    import ml_dtypes

    wflat = np.ascontiguousarray(weights, dtype=np.float32).reshape(NTOK * K)
    iflat = np.ascontiguousarray(indexes).reshape(NTOK * K).astype(np.int64)

    woutT = np.ascontiguousarray(w_out, dtype=np.float32).T       # [D, E]
    wout_host = np.ascontiguousarray(
        woutT.reshape(2, P, E).transpose(1, 0, 2).reshape(P, 2 * E)
    ).astype(ml_dtypes.bfloat16)
    bias_host = np.ascontiguousarray(
        np.broadcast_to(np.asarray(b_out, dtype=np.float32), (P, E))
    )

    plans, in_maps = [], []
    for c in range(NCORES):
        lo, hi = c * TPC * K, (c + 1) * TPC * K
        plan = _plan_core(iflat[lo:hi], wflat[lo:hi])
        nslab = sum(c["S"] for c in plan["calls"])
        plan["NSLAB"] = nslab
        tw_host = np.zeros((P, 2 * nslab), dtype=np.float32)
        idx_host = np.zeros((P, plan["XTOT"]), dtype=np.int16)
        si = 0
        for call in plan["calls"]:
            idx_host[:, call["xoff"]:call["xoff"] + call["X"]] = call["idx16"]
            for sl in call["slabs"]:
                sl["si"] = si
                tw_host[:, 2 * si] = sl["tcol"]
                tw_host[:, 2 * si + 1] = sl["wcol"]
                si += 1
        iota_host = np.broadcast_to(
            np.arange(SMAXTOK, dtype=np.float32), (P, SMAXTOK)).copy()
        plans.append(plan)
        in_maps.append({
            "idx": idx_host,
            "tw": tw_host,
            "iota": iota_host,
            "wout": wout_host,
            "bias": bias_host,
        })
    return plans, in_maps


def kernel(weights, indexes, knowledge_base, w_out, b_out):
    from concourse.bass_utils import run_bass_kernel_spmd

    kb_host = np.ascontiguousarray(knowledge_base, dtype=np.float32)
    plans, in_maps = _host_prep(weights, indexes, w_out, b_out)

    outs = []
    for c in range(NCORES):
        sig = ("v6", _plan_signature(plans[c]))
        if sig not in _CACHE:
            _CACHE[sig] = _build_bass(plans[c])
        nc = _CACHE[sig]
        if c == 0:
            _CACHE["nc"] = nc
        in_maps[c]["kb"] = kb_host
        res = run_bass_kernel_spmd(nc, [in_maps[c]], [0])
        outs.append(res.results[0]["out"].astype(np.float32))

    return np.concatenate(outs, axis=0).reshape(B, T, E)
